# revision 3
# baseline (speedup 1.0000x reference)
"""ChildSum TreeLSTM encoder kernel for Trainium2 (8 NeuronCores, SPMD).

Strategy: shard nodes (N) and edges (E) jointly by contiguous segment
ranges across 8 cores (segment_ids are sorted).  Within a core, nodes are
processed in 64 tiles of 128; each tile's child edges are padded to a
uniform number of 128-edge chunks (cmax) so all cores run one program.

All edge-sized tensors are stored fp16 (values are ~N(0,1), far from
fp16 range limits; fp16's 10-bit mantissa keeps relative error ~3e-3
where bf16 storage gave ~2e-2) and all matmuls run in fp16 (1 cycle/row
on the PE vs 4 for fp32).

Per 128-node tile t:
  fxb   = x @ W_f + b_f                       (PE, xT tiles as lhsT)
  onehotT[n,e] = (n == seg[e])                (DVE is_equal vs host-bcast seg)
  onehot[e,n]  = (seg[e] == n)                (DVE is_equal vs iota row)
  per edge chunk s:
      f_pre = onehotT_s.T @ fxb + prev_h @ U_f  (PE, PSUM accumulate)
  f = sigmoid(f_pre)                          (ACT, batched)
  fc = f * prev_c                             (Pool, batched)
  zx = x-part of [x | h_tilde] @ W_combined   (PE, fills gap while ACT/Pool run)
  acc = sum_s onehot_s.T @ [prev_h | fc]      (PE scatter, PSUM accumulate)
  h_tildeT via PE transpose of acc[:, :256]
  z += h_tilde-part                           (PE)
  c = sig(z_i)*tanh(z_u) + fc_sum ; h = sig(z_o)*tanh(c)
Outputs written fp16, upcast to fp32 on host.
"""

import numpy as np

N, E, EDIM, HDIM = 65536, 262144, 300, 256
NC = 8
NLOC = N // NC          # 8192 nodes per core
P = 128
NT = NLOC // P          # 64 node tiles per core
KX = 3                  # xT K-chunks (384 = 300 + ones-row + pad)
XPAD = KX * P           # 384
KH = HDIM // P          # 2


def _preprocess(x, prev_c, prev_h, W_combined, b_combined, W_f, U_f, b_f,
                segment_ids):
    f16 = np.float16
    seg = np.asarray(segment_ids).astype(np.int64)
    x = np.asarray(x, dtype=np.float32)
    prev_c = np.asarray(prev_c, dtype=np.float32)
    prev_h = np.asarray(prev_h, dtype=np.float32)
    W_combined = np.asarray(W_combined, dtype=np.float32)
    b_combined = np.asarray(b_combined, dtype=np.float32)
    W_f = np.asarray(W_f, dtype=np.float32)
    U_f = np.asarray(U_f, dtype=np.float32)
    b_f = np.asarray(b_f, dtype=np.float32)

    GT = N // P                          # 512 global node tiles
    starts = np.searchsorted(seg, np.arange(0, N + 1, P))
    cnt = np.diff(starts)                # edges per node tile
    cmax = int(np.ceil(cnt.max() / P))
    epc = cmax * P

    ar = np.arange(epc)
    idx = starts[:-1, None] + ar[None, :]          # [GT, epc]
    valid = ar[None, :] < cnt[:, None]             # [GT, epc]
    idxc = np.where(valid, np.minimum(idx, E - 1), 0)

    ph = prev_h[idxc].astype(f16)                  # [GT, epc, 256]
    pc = prev_c[idxc].astype(f16)
    pht = np.ascontiguousarray(ph.transpose(0, 2, 1))
    # relative segment id within the tile; invalid edges get 255 (never
    # matches iota 0..127, so they contribute nothing to any one-hot)
    segrel = np.where(valid, seg[idxc] - P * np.arange(GT)[:, None],
                      255).astype(f16)             # [GT, epc]
    segb = np.broadcast_to(segrel[:, None, :], (GT, P, epc))  # [GT, 128, epc]

    # padded weights; ones-row folds biases into the matmuls
    wf_pad = np.zeros((XPAD, HDIM), f16)
    wf_pad[:EDIM] = W_f.astype(f16)
    wf_pad[EDIM] = b_f.astype(f16)
    wc_pad = np.zeros((XPAD + HDIM, 3 * HDIM), f16)
    wc_pad[:EDIM] = W_combined[:EDIM].astype(f16)
    wc_pad[EDIM] = b_combined.astype(f16)
    wc_pad[XPAD:] = W_combined[EDIM:].astype(f16)

    in_maps = []
    for c in range(NC):
        xt = np.zeros((XPAD, NLOC), f16)
        xt[:EDIM] = x[c * NLOC:(c + 1) * NLOC].T.astype(f16)
        xt[EDIM] = 1.0
        g0, g1 = c * NT, (c + 1) * NT
        in_maps.append({
            "xt": np.ascontiguousarray(xt),
            "wf": wf_pad,
            "wc": wc_pad,
            "uf": U_f.astype(f16),
            "ph": np.ascontiguousarray(ph[g0:g1]),
            "pht": np.ascontiguousarray(pht[g0:g1]),
            "pc": np.ascontiguousarray(pc[g0:g1]),
            "segb": np.ascontiguousarray(segb[g0:g1]),
            "segc": np.ascontiguousarray(segrel[g0:g1]),
        })
    return in_maps, cmax


def _build(cmax, nt=NT):
    import concourse.bass as bass
    import concourse.mybir as mybir
    import concourse.tile as tile
    from concourse import bacc
    from concourse.masks import make_identity

    dt = mybir.dt.float32
    ht = mybir.dt.float16
    epc = cmax * P
    H2 = 2 * HDIM

    nc = bacc.Bacc("TRN2", target_bir_lowering=False, debug=False,
                   num_devices=NC)
    xt_d = nc.declare_dram_parameter("xt", [XPAD, NLOC], ht, isOutput=False)
    wf_d = nc.declare_dram_parameter("wf", [XPAD, HDIM], ht, isOutput=False)
    wc_d = nc.declare_dram_parameter("wc", [XPAD + HDIM, 3 * HDIM], ht,
                                     isOutput=False)
    uf_d = nc.declare_dram_parameter("uf", [HDIM, HDIM], ht, isOutput=False)
    ph_d = nc.declare_dram_parameter("ph", [NT, epc, HDIM], ht,
                                     isOutput=False)
    pht_d = nc.declare_dram_parameter("pht", [NT, HDIM, epc], ht,
                                      isOutput=False)
    pc_d = nc.declare_dram_parameter("pc", [NT, epc, HDIM], ht,
                                     isOutput=False)
    segb_d = nc.declare_dram_parameter("segb", [NT, P, epc], ht,
                                       isOutput=False)
    segc_d = nc.declare_dram_parameter("segc", [NT, epc], ht, isOutput=False)
    c_d = nc.declare_dram_parameter("c_out", [NLOC, HDIM], ht, isOutput=True)
    h_d = nc.declare_dram_parameter("h_out", [NLOC, HDIM], ht, isOutput=True)

    with tile.TileContext(nc) as tc:
        with (
            tc.tile_pool(name="const", bufs=1) as cpool,
            tc.tile_pool(name="inp", bufs=3) as ipool,
            tc.tile_pool(name="work", bufs=3) as wpool,
            tc.tile_pool(name="outp", bufs=3) as opool,
            tc.tile_pool(name="p_fpre", bufs=1, space="PSUM") as p_fpre,
            tc.tile_pool(name="p_acc", bufs=2, space="PSUM") as p_acc,
            tc.tile_pool(name="p_z", bufs=1, space="PSUM") as p_z,
            tc.tile_pool(name="p_small", bufs=1, space="PSUM") as p_small,
        ):
            # constants
            wf_sb = cpool.tile([P, KX, HDIM], ht)
            nc.sync.dma_start(out=wf_sb[:],
                              in_=wf_d.ap().rearrange("(k p) n -> p k n", p=P))
            wc_sb = cpool.tile([P, KX + KH, 3 * HDIM], ht)
            nc.sync.dma_start(out=wc_sb[:],
                              in_=wc_d.ap().rearrange("(k p) n -> p k n", p=P))
            uf_sb = cpool.tile([P, KH, HDIM], ht)
            nc.sync.dma_start(out=uf_sb[:],
                              in_=uf_d.ap().rearrange("(k p) n -> p k n", p=P))
            iota_i = cpool.tile([P, P], mybir.dt.int32)
            nc.gpsimd.iota(iota_i[:], pattern=[[1, P]], base=0,
                           channel_multiplier=0)
            iota_row = cpool.tile([P, 1, P], ht)
            nc.vector.tensor_copy(iota_row[:, 0, :], iota_i[:])
            iota_ci = cpool.tile([P, 1], mybir.dt.int32)
            nc.gpsimd.iota(iota_ci[:], pattern=[[1, 1]], base=0,
                           channel_multiplier=1)
            iota_col = cpool.tile([P, 1], ht)
            nc.vector.tensor_copy(iota_col[:], iota_ci[:])
            ident = cpool.tile([P, P], dt)
            make_identity(nc, ident[:])

            for t in range(nt):
                n0 = t * P
                # ---- loads ----
                xt = ipool.tile([P, KX, P], ht)
                nc.sync.dma_start(
                    out=xt[:],
                    in_=xt_d.ap().rearrange("(k p) n -> p k n", p=P)
                    [:, :, n0:n0 + P])
                rhs = ipool.tile([P, cmax, H2], ht, tag="rhs")
                nc.sync.dma_start(
                    out=rhs[:, :, 0:HDIM],
                    in_=ph_d.ap()[t].rearrange("(s p) h -> p s h", p=P))
                pht = ipool.tile([P, KH, epc], ht)
                nc.sync.dma_start(
                    out=pht[:],
                    in_=pht_d.ap()[t].rearrange("(k p) e -> p k e", p=P))
                pc = ipool.tile([P, cmax, HDIM], ht)
                nc.sync.dma_start(
                    out=pc[:],
                    in_=pc_d.ap()[t].rearrange("(s p) h -> p s h", p=P))
                segb = ipool.tile([P, epc], ht)
                nc.sync.dma_start(out=segb[:], in_=segb_d.ap()[t])
                segc = ipool.tile([P, cmax, 1], ht)
                nc.sync.dma_start(
                    out=segc[:, :, 0],
                    in_=segc_d.ap()[t].rearrange("(s p) -> p s", p=P))

                # ---- one-hots (DVE) ----
                onehotT = wpool.tile([P, epc], ht)
                nc.vector.tensor_tensor(
                    onehotT[:], iota_col[:].to_broadcast([P, epc]), segb[:],
                    op=mybir.AluOpType.is_equal)
                onehot = wpool.tile([P, cmax, P], ht)
                nc.vector.tensor_tensor(
                    onehot[:], segc[:].to_broadcast([P, cmax, P]),
                    iota_row[:].to_broadcast([P, cmax, P]),
                    op=mybir.AluOpType.is_equal)

                # ---- fxb = x @ W_f + b_f ----
                fxb_ps = p_small.tile([P, HDIM], dt, tag="small")
                for k in range(KX):
                    nc.tensor.matmul(fxb_ps[:], lhsT=xt[:, k, :],
                                     rhs=wf_sb[:, k, :],
                                     start=(k == 0), stop=(k == KX - 1))
                fxb = wpool.tile([P, HDIM], ht)
                nc.vector.tensor_copy(fxb[:], fxb_ps[:])

                # ---- f_pre per edge chunk (PE), sigmoid+fc in halves ----
                fpre = p_fpre.tile([P, cmax, HDIM], dt)
                f_sb = wpool.tile([P, cmax, HDIM], ht)
                sA = (cmax + 1) // 2          # first half: chunks [0, sA)
                for s in range(cmax):
                    nc.tensor.matmul(fpre[:, s, :],
                                     lhsT=onehotT[:, s * P:(s + 1) * P],
                                     rhs=fxb[:], start=True, stop=False)
                    for k in range(KH):
                        nc.tensor.matmul(fpre[:, s, :],
                                         lhsT=pht[:, k, s * P:(s + 1) * P],
                                         rhs=uf_sb[:, k, :],
                                         start=False, stop=(k == KH - 1))
                    if s == sA - 1:
                        nc.scalar.activation(
                            f_sb[:, 0:sA, :], fpre[:, 0:sA, :],
                            mybir.ActivationFunctionType.Sigmoid)
                        nc.gpsimd.tensor_mul(rhs[:, 0:sA, HDIM:H2],
                                             f_sb[:, 0:sA, :], pc[:, 0:sA, :])
                nc.scalar.activation(f_sb[:, sA:cmax, :], fpre[:, sA:cmax, :],
                                     mybir.ActivationFunctionType.Sigmoid)
                nc.gpsimd.tensor_mul(rhs[:, sA:cmax, HDIM:H2],
                                     f_sb[:, sA:cmax, :], pc[:, sA:cmax, :])

                # ---- z x-part (keeps PE busy while ACT/Pool produce fc) ----
                z = p_z.tile([P, 3 * HDIM], dt)
                for k in range(KX):
                    nc.tensor.matmul(z[:, 0:512], lhsT=xt[:, k, :],
                                     rhs=wc_sb[:, k, 0:512],
                                     start=(k == 0), stop=False)
                    nc.tensor.matmul(z[:, 512:768], lhsT=xt[:, k, :],
                                     rhs=wc_sb[:, k, 512:768],
                                     start=(k == 0), stop=False)

                # ---- scatter: acc = sum_s onehot_s.T @ [ph | fc] ----
                acc = p_acc.tile([P, H2], dt)
                for s in range(cmax):
                    nc.tensor.matmul(acc[:], lhsT=onehot[:, s, :],
                                     rhs=rhs[:, s, :],
                                     start=(s == 0), stop=(s == cmax - 1))

                # ---- h_tildeT (PE transpose of acc h-half) ----
                htld = wpool.tile([P, HDIM], dt)
                nc.vector.tensor_copy(htld[:], acc[:, 0:HDIM])
                htT = wpool.tile([P, KH, P], ht)
                for k in range(KH):
                    trp = p_small.tile([P, P], dt, tag="small")
                    nc.tensor.transpose(trp[:], htld[:, k * P:(k + 1) * P],
                                        ident[:])
                    nc.vector.tensor_copy(htT[:, k, :], trp[:])

                # ---- z h_tilde-part ----
                for k in range(KH):
                    nc.tensor.matmul(z[:, 0:512], lhsT=htT[:, k, :],
                                     rhs=wc_sb[:, KX + k, 0:512],
                                     start=False, stop=(k == KH - 1))
                    nc.tensor.matmul(z[:, 512:768], lhsT=htT[:, k, :],
                                     rhs=wc_sb[:, KX + k, 512:768],
                                     start=False, stop=(k == KH - 1))

                # ---- gates ----
                szio = wpool.tile([P, H2], dt)
                nc.scalar.activation(szio[:], z[:, 0:H2],
                                     mybir.ActivationFunctionType.Sigmoid)
                tzu = wpool.tile([P, HDIM], dt)
                nc.scalar.activation(tzu[:], z[:, H2:3 * HDIM],
                                     mybir.ActivationFunctionType.Tanh)
                ci = wpool.tile([P, HDIM], dt)
                nc.vector.tensor_mul(ci[:], szio[:, 0:HDIM], tzu[:])
                c_sb = opool.tile([P, HDIM], ht)
                nc.vector.tensor_add(c_sb[:], ci[:], acc[:, HDIM:H2])
                tc_sb = wpool.tile([P, HDIM], dt)
                nc.scalar.activation(tc_sb[:], c_sb[:],
                                     mybir.ActivationFunctionType.Tanh)
                h_sb = opool.tile([P, HDIM], ht)
                nc.gpsimd.tensor_mul(h_sb[:], szio[:, HDIM:H2], tc_sb[:])
                nc.sync.dma_start(out=c_d.ap()[n0:n0 + P, :], in_=c_sb[:])
                nc.sync.dma_start(out=h_d.ap()[n0:n0 + P, :], in_=h_sb[:])

    nc.compile()
    return nc


def kernel(x, prev_c, prev_h, W_combined, b_combined, W_f, U_f, b_f,
           segment_ids, _trace=False):
    from concourse.bass_utils import run_bass_kernel_spmd

    in_maps, cmax = _preprocess(x, prev_c, prev_h, W_combined, b_combined,
                                W_f, U_f, b_f, segment_ids)
    nc = _build(cmax)
    res = run_bass_kernel_spmd(nc, in_maps, list(range(NC)), trace=_trace)
    c = np.concatenate([np.asarray(res.results[i]["c_out"], np.float32)
                        for i in range(NC)], axis=0)
    h = np.concatenate([np.asarray(res.results[i]["h_out"], np.float32)
                        for i in range(NC)], axis=0)
    kernel._last_exec_time_ns = res.exec_time_ns
    return (c, h)


# revision 5
# speedup vs baseline: 1.5427x; 1.5427x over previous
"""ChildSum TreeLSTM encoder kernel for Trainium2 (8 NeuronCores, SPMD).

Strategy: shard nodes (N) and edges (E) jointly by contiguous segment
ranges across 8 cores (segment_ids are sorted).  Within a core, nodes are
processed in 64 tiles of 128; each tile's child edges are padded to a
uniform number of 128-edge chunks (cmax) so all cores run one program.

All edge-sized tensors are stored fp16 (values are ~N(0,1), far from
fp16 range limits; fp16's 10-bit mantissa keeps relative error ~3e-3
where bf16 storage gave ~2e-2) and all matmuls run in fp16 (1 cycle/row
on the PE vs 4 for fp32; LDWEIGHTS pipelines under the previous matmul).

Per 128-node tile t (PE emission order = engine order):
  fxb   = x @ W_f + b_f                        (PE)
  onehotT[n,e] = (n == seg[e])                 (DVE vs host-bcast seg row)
  onehot[e,n]  = (seg[e] == n)                 (DVE vs iota row)
  per chunk s: f_pre_s = onehotT_s.T @ fxb + prev_h @ U_f  (PE -> PSUM ring)
               f_s = sigmoid(f_pre_s) (ACT);  fc_s = f_s * prev_c_s (DVE)
  zx    = x-part of [x | h_tilde] @ W_combined (PE; fills the fc latency)
  h_tildeT[k] = sum_s ph_s[:,k].T @ onehot_s   (PE, direct transposed scatter)
  fc_sum = sum_s onehot_s.T @ fc_s             (PE)
  z += h_tildeT-part                           (PE)
  c = sig(z_i)*tanh(z_u) + fc_sum ; h = sig(z_o)*tanh(c)   (ACT/Pool)
Outputs written fp16, upcast to fp32 on host.
"""

import numpy as np

N, E, EDIM, HDIM = 65536, 262144, 300, 256
NC = 8
NLOC = N // NC          # 8192 nodes per core
P = 128
NT = NLOC // P          # 64 node tiles per core
KX = 3                  # xT K-chunks (384 = 300 + ones-row + pad)
XPAD = KX * P           # 384
KH = HDIM // P          # 2
NRING = 6               # fpre PSUM ring slots


def _preprocess(x, prev_c, prev_h, W_combined, b_combined, W_f, U_f, b_f,
                segment_ids):
    f16 = np.float16
    seg = np.asarray(segment_ids).astype(np.int64)
    x = np.asarray(x, dtype=np.float32)
    prev_c = np.asarray(prev_c, dtype=np.float32)
    prev_h = np.asarray(prev_h, dtype=np.float32)
    W_combined = np.asarray(W_combined, dtype=np.float32)
    b_combined = np.asarray(b_combined, dtype=np.float32)
    W_f = np.asarray(W_f, dtype=np.float32)
    U_f = np.asarray(U_f, dtype=np.float32)
    b_f = np.asarray(b_f, dtype=np.float32)

    GT = N // P                          # 512 global node tiles
    starts = np.searchsorted(seg, np.arange(0, N + 1, P))
    cnt = np.diff(starts)                # edges per node tile
    cmax = int(np.ceil(cnt.max() / P))
    epc = cmax * P

    ar = np.arange(epc)
    idx = starts[:-1, None] + ar[None, :]          # [GT, epc]
    valid = ar[None, :] < cnt[:, None]             # [GT, epc]
    idxc = np.where(valid, np.minimum(idx, E - 1), 0)

    ph = prev_h[idxc].astype(f16)                  # [GT, epc, 256]
    pc = prev_c[idxc].astype(f16)
    pht = np.ascontiguousarray(ph.transpose(0, 2, 1))
    # relative segment id within the tile; invalid edges get 255 (never
    # matches iota 0..127, so they contribute nothing to any one-hot)
    segrel = np.where(valid, seg[idxc] - P * np.arange(GT)[:, None],
                      255).astype(f16)             # [GT, epc]
    segb = np.broadcast_to(segrel[:, None, :], (GT, P, epc))  # [GT, 128, epc]

    # padded weights; ones-row folds biases into the matmuls
    wf_pad = np.zeros((XPAD, HDIM), f16)
    wf_pad[:EDIM] = W_f.astype(f16)
    wf_pad[EDIM] = b_f.astype(f16)
    wc_pad = np.zeros((XPAD + HDIM, 3 * HDIM), f16)
    wc_pad[:EDIM] = W_combined[:EDIM].astype(f16)
    wc_pad[EDIM] = b_combined.astype(f16)
    wc_pad[XPAD:] = W_combined[EDIM:].astype(f16)

    in_maps = []
    for c in range(NC):
        xt = np.zeros((XPAD, NLOC), f16)
        xt[:EDIM] = x[c * NLOC:(c + 1) * NLOC].T.astype(f16)
        xt[EDIM] = 1.0
        g0, g1 = c * NT, (c + 1) * NT
        in_maps.append({
            "xt": np.ascontiguousarray(xt),
            "wf": wf_pad,
            "wc": wc_pad,
            "uf": U_f.astype(f16),
            "ph": np.ascontiguousarray(ph[g0:g1]),
            "pht": np.ascontiguousarray(pht[g0:g1]),
            "pc": np.ascontiguousarray(pc[g0:g1]),
            "segb": np.ascontiguousarray(segb[g0:g1]),
            "segc": np.ascontiguousarray(segrel[g0:g1]),
        })
    return in_maps, cmax


def _build(cmax, nt=NT):
    import concourse.bass as bass
    import concourse.mybir as mybir
    import concourse.tile as tile
    from concourse import bacc

    dt = mybir.dt.float32
    ht = mybir.dt.float16
    epc = cmax * P
    H2 = 2 * HDIM

    nc = bacc.Bacc("TRN2", target_bir_lowering=False, debug=False,
                   num_devices=NC)
    xt_d = nc.declare_dram_parameter("xt", [XPAD, NLOC], ht, isOutput=False)
    wf_d = nc.declare_dram_parameter("wf", [XPAD, HDIM], ht, isOutput=False)
    wc_d = nc.declare_dram_parameter("wc", [XPAD + HDIM, 3 * HDIM], ht,
                                     isOutput=False)
    uf_d = nc.declare_dram_parameter("uf", [HDIM, HDIM], ht, isOutput=False)
    ph_d = nc.declare_dram_parameter("ph", [NT, epc, HDIM], ht,
                                     isOutput=False)
    pht_d = nc.declare_dram_parameter("pht", [NT, HDIM, epc], ht,
                                      isOutput=False)
    pc_d = nc.declare_dram_parameter("pc", [NT, epc, HDIM], ht,
                                     isOutput=False)
    segb_d = nc.declare_dram_parameter("segb", [NT, P, epc], ht,
                                       isOutput=False)
    segc_d = nc.declare_dram_parameter("segc", [NT, epc], ht, isOutput=False)
    c_d = nc.declare_dram_parameter("c_out", [NLOC, HDIM], ht, isOutput=True)
    h_d = nc.declare_dram_parameter("h_out", [NLOC, HDIM], ht, isOutput=True)

    with tile.TileContext(nc) as tc:
        with (
            tc.tile_pool(name="const", bufs=1) as cpool,
            tc.tile_pool(name="inp", bufs=3) as ipool,
            tc.tile_pool(name="work", bufs=3) as wpool,
            tc.tile_pool(name="outp", bufs=3) as opool,
            tc.tile_pool(name="p_ring", bufs=1, space="PSUM") as p_ring,
            tc.tile_pool(name="p_acc", bufs=2, space="PSUM") as p_acc,
            tc.tile_pool(name="p_z", bufs=1, space="PSUM") as p_z,
            tc.tile_pool(name="p_fxb", bufs=1, space="PSUM") as p_fxb,
        ):
            # constants
            wf_sb = cpool.tile([P, KX, HDIM], ht)
            nc.sync.dma_start(out=wf_sb[:],
                              in_=wf_d.ap().rearrange("(k p) n -> p k n", p=P))
            wc_sb = cpool.tile([P, KX + KH, 3 * HDIM], ht)
            nc.sync.dma_start(out=wc_sb[:],
                              in_=wc_d.ap().rearrange("(k p) n -> p k n", p=P))
            uf_sb = cpool.tile([P, KH, HDIM], ht)
            nc.sync.dma_start(out=uf_sb[:],
                              in_=uf_d.ap().rearrange("(k p) n -> p k n", p=P))
            iota_i = cpool.tile([P, P], mybir.dt.int32)
            nc.gpsimd.iota(iota_i[:], pattern=[[1, P]], base=0,
                           channel_multiplier=0)
            iota_row = cpool.tile([P, 1, P], ht)
            nc.vector.tensor_copy(iota_row[:, 0, :], iota_i[:])
            iota_ci = cpool.tile([P, 1], mybir.dt.int32)
            nc.gpsimd.iota(iota_ci[:], pattern=[[1, 1]], base=0,
                           channel_multiplier=1)
            iota_col = cpool.tile([P, 1], ht)
            nc.vector.tensor_copy(iota_col[:], iota_ci[:])

            # fpre PSUM ring shared across tiles (6 slots x [128,256] f32)
            fring = p_ring.tile([P, NRING, HDIM], dt)

            for t in range(nt):
                n0 = t * P
                # ---- loads ----
                xt = ipool.tile([P, KX, P], ht)
                nc.sync.dma_start(
                    out=xt[:],
                    in_=xt_d.ap().rearrange("(k p) n -> p k n", p=P)
                    [:, :, n0:n0 + P])
                ph = ipool.tile([P, cmax, HDIM], ht)
                nc.sync.dma_start(
                    out=ph[:],
                    in_=ph_d.ap()[t].rearrange("(s p) h -> p s h", p=P))
                pht = ipool.tile([P, KH, epc], ht)
                nc.sync.dma_start(
                    out=pht[:],
                    in_=pht_d.ap()[t].rearrange("(k p) e -> p k e", p=P))
                pc = ipool.tile([P, cmax, HDIM], ht)
                nc.sync.dma_start(
                    out=pc[:],
                    in_=pc_d.ap()[t].rearrange("(s p) h -> p s h", p=P))
                segb = ipool.tile([P, epc], ht)
                nc.sync.dma_start(out=segb[:], in_=segb_d.ap()[t])
                segc = ipool.tile([P, cmax, 1], ht)
                nc.sync.dma_start(
                    out=segc[:, :, 0],
                    in_=segc_d.ap()[t].rearrange("(s p) -> p s", p=P))

                # ---- one-hots (DVE) ----
                onehotT = wpool.tile([P, epc], ht)
                nc.vector.tensor_tensor(
                    onehotT[:], iota_col[:].to_broadcast([P, epc]), segb[:],
                    op=mybir.AluOpType.is_equal)
                onehot = wpool.tile([P, cmax, P], ht)
                nc.vector.tensor_tensor(
                    onehot[:], segc[:].to_broadcast([P, cmax, P]),
                    iota_row[:].to_broadcast([P, cmax, P]),
                    op=mybir.AluOpType.is_equal)

                # ---- fxb = x @ W_f + b_f ----
                fxb_ps = p_fxb.tile([P, HDIM], dt)
                for k in range(KX):
                    nc.tensor.matmul(fxb_ps[:], lhsT=xt[:, k, :],
                                     rhs=wf_sb[:, k, :],
                                     start=(k == 0), stop=(k == KX - 1))
                fxb = wpool.tile([P, HDIM], ht)
                nc.vector.tensor_copy(fxb[:], fxb_ps[:])

                # ---- f_pre per chunk -> sigmoid (ACT) -> fc (DVE) ----
                f_sb = wpool.tile([P, cmax, HDIM], ht)
                fc_sb = wpool.tile([P, cmax, HDIM], ht)
                for s in range(cmax):
                    r = (t * cmax + s) % NRING
                    nc.tensor.matmul(fring[:, r, :],
                                     lhsT=onehotT[:, s * P:(s + 1) * P],
                                     rhs=fxb[:], start=True, stop=False)
                    for k in range(KH):
                        nc.tensor.matmul(fring[:, r, :],
                                         lhsT=pht[:, k, s * P:(s + 1) * P],
                                         rhs=uf_sb[:, k, :],
                                         start=False, stop=(k == KH - 1))
                    nc.scalar.activation(f_sb[:, s, :], fring[:, r, :],
                                         mybir.ActivationFunctionType.Sigmoid)
                    nc.vector.tensor_mul(fc_sb[:, s, :], f_sb[:, s, :],
                                         pc[:, s, :])

                # ---- z x-part (keeps PE busy while ACT/DVE produce fc) ----
                z = p_z.tile([P, 3 * HDIM], dt)
                for k in range(KX):
                    nc.tensor.matmul(z[:, 0:512], lhsT=xt[:, k, :],
                                     rhs=wc_sb[:, k, 0:512],
                                     start=(k == 0), stop=False)
                    nc.tensor.matmul(z[:, 512:768], lhsT=xt[:, k, :],
                                     rhs=wc_sb[:, k, 512:768],
                                     start=(k == 0), stop=False)

                # ---- scatters into acc: fc_sum [0:256], h_tildeT [256:512]
                acc = p_acc.tile([P, H2], dt)
                for k in range(KH):
                    for s in range(cmax):
                        nc.tensor.matmul(
                            acc[:, HDIM + k * P:HDIM + (k + 1) * P],
                            lhsT=ph[:, s, k * P:(k + 1) * P],
                            rhs=onehot[:, s, :],
                            start=(s == 0), stop=(s == cmax - 1))
                for s in range(cmax):
                    nc.tensor.matmul(acc[:, 0:HDIM], lhsT=onehot[:, s, :],
                                     rhs=fc_sb[:, s, :],
                                     start=(s == 0), stop=(s == cmax - 1))

                # ---- h_tildeT to SBUF, z h-part ----
                htT = wpool.tile([P, KH, P], ht)
                nc.vector.tensor_copy(htT[:], acc[:, HDIM:H2])
                for k in range(KH):
                    nc.tensor.matmul(z[:, 0:512], lhsT=htT[:, k, :],
                                     rhs=wc_sb[:, KX + k, 0:512],
                                     start=False, stop=(k == KH - 1))
                    nc.tensor.matmul(z[:, 512:768], lhsT=htT[:, k, :],
                                     rhs=wc_sb[:, KX + k, 512:768],
                                     start=False, stop=(k == KH - 1))

                # ---- gates (ACT + Pool, off the PE critical path) ----
                szio = wpool.tile([P, H2], dt)
                nc.scalar.activation(szio[:], z[:, 0:H2],
                                     mybir.ActivationFunctionType.Sigmoid)
                tzu = wpool.tile([P, HDIM], dt)
                nc.scalar.activation(tzu[:], z[:, H2:3 * HDIM],
                                     mybir.ActivationFunctionType.Tanh)
                ci = wpool.tile([P, HDIM], dt)
                nc.gpsimd.tensor_mul(ci[:], szio[:, 0:HDIM], tzu[:])
                c_sb = opool.tile([P, HDIM], ht)
                nc.vector.tensor_add(c_sb[:], ci[:], acc[:, 0:HDIM])
                tc_sb = wpool.tile([P, HDIM], dt)
                nc.scalar.activation(tc_sb[:], c_sb[:],
                                     mybir.ActivationFunctionType.Tanh)
                h_sb = opool.tile([P, HDIM], ht)
                nc.gpsimd.tensor_mul(h_sb[:], szio[:, HDIM:H2], tc_sb[:])
                nc.sync.dma_start(out=c_d.ap()[n0:n0 + P, :], in_=c_sb[:])
                nc.sync.dma_start(out=h_d.ap()[n0:n0 + P, :], in_=h_sb[:])

    nc.compile()
    return nc


def kernel(x, prev_c, prev_h, W_combined, b_combined, W_f, U_f, b_f,
           segment_ids, _trace=False):
    from concourse.bass_utils import run_bass_kernel_spmd

    in_maps, cmax = _preprocess(x, prev_c, prev_h, W_combined, b_combined,
                                W_f, U_f, b_f, segment_ids)
    nc = _build(cmax)
    res = run_bass_kernel_spmd(nc, in_maps, list(range(NC)), trace=_trace)
    c = np.concatenate([np.asarray(res.results[i]["c_out"], np.float32)
                        for i in range(NC)], axis=0)
    h = np.concatenate([np.asarray(res.results[i]["h_out"], np.float32)
                        for i in range(NC)], axis=0)
    kernel._last_exec_time_ns = res.exec_time_ns
    return (c, h)
